# revision 1
# baseline (speedup 1.0000x reference)
"""MI-LSTM (attention LSTM) + LSTM + linear head for Trainium2, 8-core batch-parallel.

v7: per-tau software-pipelined chains. Per 128-row batch tile the
recurrence loop is PE (transpose+gates) -> ScalarE (activations) ->
VectorE (candidate products, fold-tree reductions) -> GpSimd (weighted
candidate sum, state update) -> ScalarE (tanh c') -> GpSimd (h') and the
two tiles plus the phase-2 LSTM chain interleave in the gaps. Inputs are
host-pre-transposed to feature-major bf16 and DMA'd straight into the
matmul lhsT tile. Phase-2 bias rides an extra ones-row of the h1 history.
GpSimd touches only SBUF (hardware rule).
"""

import os
import numpy as np
import ml_dtypes

import concourse.bacc as bacc
import concourse.bass as bass
import concourse.mybir as mybir
from concourse.tile import TileContext
from concourse.bass_utils import run_bass_kernel_spmd

F32 = mybir.dt.float32
BF16 = mybir.dt.bfloat16
ALU = mybir.AluOpType
ACTF = mybir.ActivationFunctionType
AX = mybir.AxisListType

S, B, F, H, K = 256, 2048, 5, 64, 8
NC = 8
BL = B // NC
NT = BL // 128
NCAND = K + 1
NSIG = NCAND * H + 2 * H   # 704
NTAN = NCAND * H           # 576
GSTR = 1536

LAST_RESULTS = {}


def _build(n_steps: int, b_att: float):
    nc = bacc.Bacc(None, target_bir_lowering=False)

    xin = nc.dram_tensor("xin", [n_steps, NT, 45, 128], BF16, kind="ExternalInput")
    wall = nc.dram_tensor("wall", [128, 1280], BF16, kind="ExternalInput")
    watt = nc.dram_tensor("watt", [H, H], BF16, kind="ExternalInput")
    wca = nc.dram_tensor("wca", [H + 1, 4 * H], BF16, kind="ExternalInput")
    wcb = nc.dram_tensor("wcb", [H, 4 * H], BF16, kind="ExternalInput")
    linw = nc.dram_tensor("linw", [128, H], BF16, kind="ExternalInput")
    onesrow = nc.dram_tensor("onesrow", [1, n_steps * BL], BF16, kind="ExternalInput")
    onesr2 = nc.dram_tensor("onesr2", [1, 128], BF16, kind="ExternalInput")
    idf32 = nc.dram_tensor("idf32", [128, 128], F32, kind="ExternalInput")
    idb16 = nc.dram_tensor("idb16", [128, 128], BF16, kind="ExternalInput")
    out = nc.dram_tensor("out", [n_steps, BL, 1], F32, kind="ExternalOutput")

    with TileContext(nc) as tc:
        with (
            tc.tile_pool(name="state", bufs=1) as st,
            tc.tile_pool(name="wts", bufs=1) as wp,
            tc.tile_pool(name="work", bufs=2) as wk,
            tc.tile_pool(name="gpsum", bufs=1, space="PSUM") as gp,
            tc.tile_pool(name="mpsum", bufs=1, space="PSUM") as mp,
            tc.tile_pool(name="g2psum", bufs=1, space="PSUM") as g2p,
        ):
            W = wp.tile([128, 1280], BF16, tag="wall")
            WA = wp.tile([H, H], BF16, tag="watt")
            WCA = wp.tile([H + 1, 4 * H], BF16, tag="wca")
            WCB = wp.tile([H, 4 * H], BF16, tag="wcb")
            LW = wp.tile([128, H], BF16, tag="linw")
            IDF = wp.tile([128, 128], F32, tag="idf32")
            IDB = wp.tile([128, 128], BF16, tag="idb16")
            for t_, d_ in ((W, wall), (WA, watt), (WCA, wca), (WCB, wcb),
                           (LW, linw), (IDF, idf32), (IDB, idb16)):
                nc.sync.dma_start(t_[:], d_[:])

            XY0 = st.tile([128, 128], BF16, tag="xyht0")
            XY1 = st.tile([128, 128], BF16, tag="xyht1")
            XYT = (XY0, XY1)
            HC1 = st.tile([128, NT * 128], F32, tag="hc1")
            CT = st.tile([H, BL], BF16, tag="ct")
            VV = st.tile([128, NT * H], BF16, tag="vv")
            HST = st.tile([H + 1, n_steps * BL], BF16, tag="hst")
            C2 = st.tile([128, NT * H], F32, tag="c2")
            H2B = st.tile([128, NT * H], BF16, tag="h2b")
            H2T = st.tile([H, BL], BF16, tag="h2t")
            OACC = st.tile([128, NT * n_steps], F32, tag="oacc")

            for tau in range(NT):
                nc.vector.memset(XYT[tau][32:64, :], 0.0)
                nc.vector.memset(XYT[tau][64:128, :], 0.0)
                nc.sync.dma_start(XYT[tau][45:46, :], onesr2[:])
            nc.vector.memset(HC1[:], 0.0)
            nc.vector.memset(CT[:], 0.0)
            nc.vector.memset(C2[:], 0.0)
            nc.vector.memset(H2B[:], 0.0)
            nc.vector.memset(H2T[:], 0.0)
            nc.sync.dma_start(HST[H:H + 1, :], onesrow[:])

            def p2_step(j, misc):
                G2 = g2p.tile([128, 512], F32, tag="g2")
                for tau in range(NT):
                    o0 = tau * 256
                    c0 = (j * NT + tau) * 128
                    nc.tensor.matmul(G2[:, o0:o0 + 256],
                                     HST[:, c0:c0 + 128], WCA[:],
                                     start=True, stop=False)
                    nc.tensor.matmul(G2[:, o0:o0 + 256],
                                     H2T[:, tau * 128:(tau + 1) * 128],
                                     WCB[:], start=False, stop=True)
                g2v = G2[:].rearrange("p (t c) -> p t c", t=NT)
                S2 = wk.tile([128, NT * 192], BF16, tag="s2")
                s2v = S2[:].rearrange("p (t c) -> p t c", t=NT)
                G2T = wk.tile([128, NT * H], BF16, tag="g2t")
                nc.scalar.activation(s2v, g2v[:, :, 0:192], ACTF.Sigmoid)
                nc.scalar.activation(
                    G2T[:].rearrange("p (t c) -> p t c", t=NT),
                    g2v[:, :, 192:256], ACTF.Tanh)

                IG = wk.tile([128, NT * H], BF16, tag="ig")
                nc.gpsimd.tensor_tensor(
                    IG[:].rearrange("p (t h) -> p t h", t=NT),
                    s2v[:, :, 0:64],
                    G2T[:].rearrange("p (t h) -> p t h", t=NT), ALU.mult)
                FC2 = wk.tile([128, NT * H], F32, tag="fc2")
                nc.gpsimd.tensor_tensor(
                    FC2[:].rearrange("p (t h) -> p t h", t=NT),
                    s2v[:, :, 64:128],
                    C2[:].rearrange("p (t h) -> p t h", t=NT), ALU.mult)
                nc.gpsimd.tensor_tensor(C2[:], FC2[:], IG[:], ALU.add)
                TC2 = wk.tile([128, NT * H], BF16, tag="tc2")
                nc.scalar.activation(
                    TC2[:].rearrange("p (t h) -> p t h", t=NT),
                    C2[:].rearrange("p (t h) -> p t h", t=NT), ACTF.Tanh)
                nc.gpsimd.tensor_tensor(
                    H2B[:].rearrange("p (t h) -> p t h", t=NT),
                    s2v[:, :, 128:192],
                    TC2[:].rearrange("p (t h) -> p t h", t=NT), ALU.mult)
                h2p = misc[0:64, 384:512].bitcast(BF16)  # [64, 256] bf16
                for tau in range(NT):
                    nc.tensor.transpose(h2p[:, tau * 128:(tau + 1) * 128],
                                        H2B[:, tau * H:(tau + 1) * H], IDB[:])
                    ZZ = wk.tile([128, H], BF16, tag="zz")
                    nc.vector.scalar_tensor_tensor(
                        ZZ[:], H2B[:, tau * H:(tau + 1) * H], 0.0, LW[:],
                        ALU.max, ALU.mult,
                        accum_out=OACC[:, tau * n_steps + j:tau * n_steps + j + 1])
                nc.vector.tensor_copy(H2T[:], h2p)

            # per-step work tiles are allocated fresh (pool rotates 2 bufs)
            for t in range(n_steps):
                misc = mp.tile([128, 512], F32, tag="misc")

                GP0 = gp.tile([128, 1280], F32, tag="gates0")
                GP1 = gp.tile([128, 1280], F32, tag="gates1")
                GPT = (GP0, GP1)
                SG = wk.tile([128, NT * NSIG], BF16, tag="sg")
                TH = wk.tile([128, NT * NTAN], BF16, tag="th")
                LT = wk.tile([128, NT * NTAN], BF16, tag="lt")
                Z = wk.tile([128, NT * NTAN], BF16, tag="z")
                UF = wk.tile([128, NT * NCAND * 32], BF16, tag="uf")
                U = wk.tile([128, NT * NCAND], F32, tag="u")
                FC = wk.tile([128, NT * H], F32, tag="fc")
                AW = wk.tile([128, NT * NTAN], BF16, tag="aw")
                LSUM = wk.tile([128, NT * H], F32, tag="lsum")
                TC1 = wk.tile([128, NT * H], BF16, tag="tc1")

                def head(tau):
                    # state transpose, copies, matmuls
                    ms = misc[:, tau * 128:(tau + 1) * 128]
                    nc.tensor.transpose(ms[0:128], HC1[:, tau * 128:(tau + 1) * 128],
                                        IDF[:])
                    nc.vector.tensor_copy(XYT[tau][64:128, :], ms[0:64])
                    nc.scalar.copy(CT[:, tau * 128:(tau + 1) * 128],
                                   ms[64:128])
                    if t > 0:
                        j0 = ((t - 1) * NT + tau) * 128
                        nc.gpsimd.tensor_copy(HST[0:64, j0:j0 + 128],
                                              XYT[tau][64:128, :])
                    nc.sync.dma_start(XYT[tau][0:45, :], xin[t, tau])
                    GPt = GPT[tau]
                    nc.tensor.matmul(GPt[:, 0:512], XYT[tau][:],
                                     W[:, 0:512], start=True, stop=True)
                    nc.tensor.matmul(GPt[:, 512:1024], XYT[tau][:],
                                     W[:, 512:1024], start=True, stop=True)
                    nc.tensor.matmul(GPt[:, 1024:1280], XYT[tau][:],
                                     W[:, 1024:1280], start=True, stop=True)
                    nc.tensor.matmul(misc[:, 256 + tau * 64:256 + (tau + 1) * 64],
                                     CT[:, tau * 128:(tau + 1) * 128], WA[:],
                                     start=True, stop=True)

                def acts(tau):
                    GPt = GPT[tau]
                    nc.scalar.activation(SG[:, tau * NSIG:(tau + 1) * NSIG],
                                         GPt[:, 0:NSIG], ACTF.Sigmoid)
                    nc.scalar.activation(TH[:, tau * NTAN:(tau + 1) * NTAN],
                                         GPt[:, NSIG:1280], ACTF.Tanh)

                def zu(tau):
                    nc.vector.tensor_copy(VV[:, tau * H:(tau + 1) * H],
                                          misc[:, 256 + tau * 64:256 + (tau + 1) * 64])
                    nc.vector.tensor_tensor(
                        LT[:, tau * NTAN:(tau + 1) * NTAN],
                        SG[:, tau * NSIG:tau * NSIG + NTAN],
                        TH[:, tau * NTAN:(tau + 1) * NTAN], ALU.mult)
                    nc.vector.tensor_tensor(
                        Z[:, tau * NTAN:(tau + 1) * NTAN]
                        .rearrange("p (k h) -> p k h", k=NCAND),
                        LT[:, tau * NTAN:(tau + 1) * NTAN]
                        .rearrange("p (k h) -> p k h", k=NCAND),
                        (VV[:, tau * H:(tau + 1) * H].unsqueeze(1)
                         .broadcast_to((128, NCAND, H))),
                        ALU.mult)
                    zv = (Z[:, tau * NTAN:(tau + 1) * NTAN]
                          .rearrange("p (k h) -> p k h", k=NCAND))
                    ufv = (UF[:, tau * NCAND * 32:(tau + 1) * NCAND * 32]
                           .rearrange("p (k h) -> p k h", k=NCAND))
                    nc.vector.tensor_tensor(ufv, zv[:, :, 0:32], zv[:, :, 32:64],
                                            ALU.add)
                    nc.vector.tensor_reduce(
                        U[:, tau * NCAND:(tau + 1) * NCAND], ufv, AX.X, ALU.add)
                    # f*c on GpSimd, off the critical chain
                    nc.gpsimd.tensor_tensor(
                        FC[:, tau * H:(tau + 1) * H],
                        SG[:, tau * NSIG + NTAN:tau * NSIG + NTAN + 64],
                        HC1[:, tau * 128 + 64:tau * 128 + 128], ALU.mult)

                UT = wk.tile([128, NT * NCAND], F32, tag="ut")
                T2 = wk.tile([128, NT * NCAND], F32, tag="t2")
                Q = wk.tile([128, NT * NCAND], F32, tag="q")
                RQ = wk.tile([128, NT * NCAND], F32, tag="rq")
                R = wk.tile([128, NT * NCAND], BF16, tag="r")
                SRED = wk.tile([128, NT], F32, tag="sred")
                RS = wk.tile([128, NT], F32, tag="rs")

                def softmax_tail(tau):
                    k0 = tau * NCAND
                    k1 = (tau + 1) * NCAND
                    nc.scalar.activation(UT[:, k0:k1], U[:, k0:k1],
                                         ACTF.Tanh, bias=b_att)
                    nc.scalar.activation(T2[:, k0:k1], UT[:, k0:k1],
                                         ACTF.Tanh, scale=0.5)
                    nc.vector.tensor_scalar(Q[:, k0:k1], T2[:, k0:k1],
                                            -1.0, 1.0, ALU.mult, ALU.add)
                    nc.vector.reciprocal_approx_fast(RQ[:, k0:k1], Q[:, k0:k1])
                    nc.vector.scalar_tensor_tensor(
                        R[:, k0:k1], T2[:, k0:k1], 1.0, RQ[:, k0:k1],
                        ALU.add, ALU.mult,
                        accum_out=SRED[:, tau:tau + 1])
                    nc.vector.reciprocal_approx_fast(RS[:, tau:tau + 1],
                                                     SRED[:, tau:tau + 1])

                def tail(tau):
                    nc.gpsimd.tensor_tensor(
                        AW[:, tau * NTAN:(tau + 1) * NTAN]
                        .rearrange("p (k h) -> p k h", k=NCAND),
                        LT[:, tau * NTAN:(tau + 1) * NTAN]
                        .rearrange("p (k h) -> p k h", k=NCAND),
                        (R[:, tau * NCAND:(tau + 1) * NCAND].unsqueeze(2)
                         .broadcast_to((128, NCAND, H))),
                        ALU.mult)
                    nc.vector.tensor_reduce(
                        LSUM[:, tau * H:(tau + 1) * H],
                        AW[:, tau * NTAN:(tau + 1) * NTAN]
                        .rearrange("p (k h) -> p h k", k=NCAND),
                        AX.X, ALU.add)
                    # c' = Lsum*rs + f*c  (VectorE), tanh (ScalarE), h' (GpSimd)
                    nc.vector.scalar_tensor_tensor(
                        HC1[:, tau * 128 + 64:tau * 128 + 128],
                        LSUM[:, tau * H:(tau + 1) * H], RS[:, tau:tau + 1],
                        FC[:, tau * H:(tau + 1) * H], ALU.mult, ALU.add)
                    nc.scalar.activation(TC1[:, tau * H:(tau + 1) * H],
                                         HC1[:, tau * 128 + 64:tau * 128 + 128],
                                         ACTF.Tanh)
                    nc.gpsimd.tensor_tensor(
                        HC1[:, tau * 128:tau * 128 + 64],
                        SG[:, tau * NSIG + NTAN + 64:tau * NSIG + NTAN + 128],
                        TC1[:, tau * H:(tau + 1) * H], ALU.mult)

                head(0)
                acts(0)
                head(1)
                zu(0)
                acts(1)
                zu(1)
                softmax_tail(0)
                softmax_tail(1)
                tail(0)
                tail(1)
                if t > 0:
                    p2_step(t - 1, misc)

            # epilogue: final h1 into HST, then last phase-2 step
            misc = mp.tile([128, 512], F32, tag="misc")
            for tau in range(NT):
                ms = misc[:, tau * 128:(tau + 1) * 128]
                nc.tensor.transpose(ms[0:128], HC1[:, tau * 128:(tau + 1) * 128],
                                    IDF[:])
                j0 = ((n_steps - 1) * NT + tau) * 128
                nc.vector.tensor_copy(HST[0:64, j0:j0 + 128], ms[0:64])
            p2_step(n_steps - 1, misc)

            ov = out.rearrange("s (tau p) o -> tau p (s o)", p=128)
            for tau in range(NT):
                nc.sync.dma_start(
                    ov[tau], OACC[:, tau * n_steps:(tau + 1) * n_steps])

    nc.finalize()
    return nc


def _prep_weights(inp):
    f32 = np.float32
    W_main, U_main, b_main = (np.asarray(inp["W_main"], f32),
                              np.asarray(inp["U_main"], f32),
                              np.asarray(inp["b_main"], f32))
    W_aux, U_aux, b_aux = (np.asarray(inp["W_aux"], f32),
                           np.asarray(inp["U_aux"], f32),
                           np.asarray(inp["b_aux"], f32))
    wall = np.zeros((128, 1280), f32)
    wall[0:5, 0:64] = W_main[:, 0:64]
    wall[64:128, 0:64] = U_main[:, 0:64]
    wall[45, 0:64] = b_main[0:64]
    for k in range(K):
        c = 64 * (k + 1)
        wall[5 + 5 * k:10 + 5 * k, c:c + 64] = W_aux[k, :, 0:64]
        wall[64:128, c:c + 64] = U_aux[k, :, 0:64]
        wall[45, c:c + 64] = b_aux[k, 0:64]
    wall[0:5, 576:640] = W_main[:, 64:128]
    wall[64:128, 576:640] = U_main[:, 64:128]
    wall[45, 576:640] = b_main[64:128]
    wall[0:5, 640:704] = W_main[:, 128:192]
    wall[64:128, 640:704] = U_main[:, 128:192]
    wall[45, 640:704] = b_main[128:192]
    wall[0:5, 704:768] = W_main[:, 192:256]
    wall[64:128, 704:768] = U_main[:, 192:256]
    wall[45, 704:768] = b_main[192:256]
    for k in range(K):
        c = 768 + 64 * k
        wall[5 + 5 * k:10 + 5 * k, c:c + 64] = W_aux[k, :, 64:128]
        wall[64:128, c:c + 64] = U_aux[k, :, 64:128]
        wall[45, c:c + 64] = b_aux[k, 64:128]

    watt = np.asarray(inp["W_att"], f32).T.copy()
    perm = np.concatenate([np.arange(0, 128), np.arange(192, 256),
                           np.arange(128, 192)])
    wca = np.zeros((H + 1, 4 * H), f32)
    wca[0:H] = np.asarray(inp["W_ih"], f32).T[:, perm]
    wca[H] = (np.asarray(inp["b_ih"], f32) + np.asarray(inp["b_hh"], f32))[perm]
    wcb = np.asarray(inp["W_hh"], f32).T[:, perm].copy()
    linw = np.broadcast_to(np.asarray(inp["lin_W"], f32), (128, H)).copy()

    bf = ml_dtypes.bfloat16
    return dict(
        wall=wall.astype(bf), watt=watt.astype(bf),
        wca=wca.astype(bf), wcb=wcb.astype(bf),
        linw=linw.astype(bf),
        onesr2=np.ones((1, 128), bf),
        idf32=np.eye(128, dtype=f32),
        idb16=np.eye(128, dtype=f32).astype(bf),
    )


def kernel(**inputs) -> np.ndarray:
    n_steps = int(os.environ.get("KERNEL_STEPS", S))
    names = ["Y"] + ["x%d" % i for i in range(1, 9)]
    big = np.stack([np.asarray(inputs[n], np.float32)[:n_steps] for n in names],
                   axis=1)
    wmaps = _prep_weights(inputs)
    wmaps["onesrow"] = np.ones((1, n_steps * BL), ml_dtypes.bfloat16)
    b_att = float(np.asarray(inputs["b_att"]).reshape(-1)[0])
    lin_b = float(np.asarray(inputs["lin_b"]).reshape(-1)[0])

    nc = _build(n_steps, b_att)
    in_maps = []
    for c in range(NC):
        m = dict(wmaps)
        sl = big[:, :, c * BL:(c + 1) * BL, :]
        ft = sl.transpose(0, 1, 3, 2).reshape(n_steps, 45, NT, 128)
        m["xin"] = np.ascontiguousarray(
            ft.transpose(0, 2, 1, 3)).astype(ml_dtypes.bfloat16)
        in_maps.append(m)

    trace = bool(int(os.environ.get("KERNEL_TRACE", "0")))
    res = run_bass_kernel_spmd(nc, in_maps, core_ids=list(range(NC)),
                               trace=trace)
    LAST_RESULTS["exec_time_ns"] = res.exec_time_ns
    LAST_RESULTS["trace"] = res.instructions_and_trace

    outs = [r["out"] for r in res.results]
    full = np.concatenate(outs, axis=1) + lin_b
    return full.astype(np.float32)

